# revision 15
# baseline (speedup 1.0000x reference)
"""Trainium2 Bass kernel for a 16-head attention block (d_model=1024, seq=4096).

Sharding: tensor-parallel over heads. Each of the 8 cores computes QKV
projections, RMSNorm(q,k), full softmax(QK^T)V attention for its 2 heads,
and a partial O-projection (its heads' slice of the contraction). The host
sums the 8 partial outputs and adds the output bias (the all-reduce of the
TP decomposition, done at unshard time).

Per-core dataflow (all matmuls fp32r, contraction always 128):
  phase 1: qkv[s,f] = xT_aug^T @ WqkvT_aug (bias via augmented ones row),
           RMSNorm over d_head on the [s_part, d_free] layout,
           PE-transpose q_hat/k_hat per head into [d(pad 128), s] tiles,
           V' = [V | 1] chunks for the fused denominator.
  phase 2: per (head, q-tile 512): S^T[k,q] blocks via k_hatT.T @ q_hatT,
           exp on ACT, z'^T[65,q] += V'^T @ probs accumulated over k-blocks
           (row 64 = softmax denominators), reciprocal + ones-row broadcast
           matmul + multiply -> z_nT[d_local 128, s].
  phase 3: out[s,dm] partial = z_nT.T @ WoT slice, PSUM->HBM direct.
"""

import numpy as np
from contextlib import ExitStack

import concourse.bass as bass
import concourse.tile as tile
from concourse import mybir
from concourse.masks import make_identity

F32 = mybir.dt.float32
F32R = mybir.dt.float32r
AF = mybir.ActivationFunctionType
ALU = mybir.AluOpType

D_MODEL = 1024
SEQ = 4096
N_HEADS = 16
D_HEAD = 64
N_CORES = 8
HEADS_LOCAL = 2
P = 128
F_LOCAL = 3 * HEADS_LOCAL * D_HEAD       # 384: [q0|q1|k0|k1|v0|v1]
DM_CHUNKS = D_MODEL // P                 # 8
DM_AUG = D_MODEL + P                     # 1152 rows: x^T plus ones-row block
SB = SEQ // P                            # 32 s-blocks
QT = 8                                   # q-tiles of 512
QW = SEQ // QT                           # 512
KB = SEQ // P                            # 32 k-blocks
EPS = 1e-6


class _SplitWaitTileContext(tile.TileContext):
    """TileContext whose exit drain splits its semaphore waits across
    single-wait sync nops: this walrus rejects CTRL instructions carrying
    more than one sync-wait command."""

    MAX_CTRL_WAITS = 1

    def _drain_and_barrier(self, tick_clock, wait_clock):
        from concourse.vector_clock import ScopedClock

        nc = self.nc
        carrier = nc.sync.nop(nofuse=True)
        wait_clock.add_sem_waits(
            carrier.ins, ScopedClock({None: tick_clock.global_clock})
        )
        si = carrier.ins.sync_info
        waits = list(si.on_wait) if si is not None and si.on_wait else []
        if len(waits) > self.MAX_CTRL_WAITS:
            sic = type(si)
            carrier.ins.sync_info = sic(
                on_wait=waits[: self.MAX_CTRL_WAITS], on_update=si.on_update
            )
            for i in range(self.MAX_CTRL_WAITS, len(waits), self.MAX_CTRL_WAITS):
                chunk = waits[i : i + self.MAX_CTRL_WAITS]
                w2 = nc.sync.nop(nofuse=True)
                si2 = w2.ins.sync_info
                w2.ins.sync_info = sic(
                    on_wait=chunk,
                    on_update=si2.on_update if si2 is not None else [],
                )
        nc.sync.drain()
        nc.all_engine_barrier()
        popped = nc._tile_sem_poison_stack.pop()
        assert popped is self._sem_poison
        nc.clear_and_free_semaphores(list(self.sems.allocated().values()))
        nc.all_engine_barrier()


def r32(ap):
    return ap.bitcast(F32R)


MAX_WAITS = 1


def _split_excess_waits(nc):
    """This walrus build rejects instructions carrying more than one or two
    sync-wait commands (CTRL and pseudo-DMA structs especially). Rewrite every
    instruction with more than MAX_WAITS waits into a chain of same-engine
    NoOps each carrying MAX_WAITS waits, followed by the original."""
    import bass_rust

    n_new = 0
    for f in nc.m.functions:
        for bb in f.blocks:
            changed = False
            out = []
            for ins in bb.instructions:
                si = ins.sync_info
                waits = list(si.on_wait) if si is not None and si.on_wait else []
                if len(waits) > MAX_WAITS:
                    changed = True
                    ncar = len(waits) - MAX_WAITS
                    for i in range(0, ncar, MAX_WAITS):
                        chunk = waits[i : min(i + MAX_WAITS, ncar)]
                        nop = mybir.InstNoOp(
                            name=f"{ins.name}-wsplit{i}", ins=[], outs=[]
                        )
                        nop.engine = ins.engine
                        nop.sync_info = bass_rust.SyncInfo(
                            on_wait=chunk, on_update=[]
                        )
                        out.append(nop)
                        n_new += 1
                    ins.sync_info = bass_rust.SyncInfo(
                        on_wait=waits[ncar:], on_update=si.on_update
                    )
                out.append(ins)
            if changed:
                bb.instructions = out
    return n_new


def build_core_kernel(split_waits=True):
    nc = bass.Bass()
    xta = nc.declare_dram_parameter("xta", [DM_AUG, SEQ], F32, isOutput=False)
    wqkvt = nc.declare_dram_parameter("wqkvt", [DM_AUG, F_LOCAL], F32, isOutput=False)
    wot = nc.declare_dram_parameter("wot", [P, D_MODEL], F32, isOutput=False)
    wqwk = nc.declare_dram_parameter("wqwk", [D_HEAD, 2], F32, isOutput=False)
    out = nc.declare_dram_parameter("out", [SEQ, D_MODEL], F32, isOutput=True)

    xta_r = xta.rearrange("(c p) s -> p c s", p=P)       # [128, 9, 4096]
    wqkvt_r = wqkvt.rearrange("(c p) f -> p c f", p=P)   # [128, 9, 384]

    with ExitStack() as ctx:
        tc = ctx.enter_context(tile.TileContext(nc))

        const = ctx.enter_context(tc.tile_pool(name="const", bufs=1))
        persist = ctx.enter_context(tc.tile_pool(name="persist", bufs=1))

        ident = const.tile([P, P], F32)
        make_identity(nc, ident)
        # fp32 constant sources (memset cannot target fp32r, and fp32r matmul
        # operands must be produced by rounding compute ops)
        zeros_f32 = const.tile([P, QW], F32)
        nc.gpsimd.memset(zeros_f32[:], 0.0)
        ones_f32 = const.tile([P, D_HEAD], F32)
        nc.gpsimd.memset(ones_f32[:], 1.0)
        onesrow_st = const.tile([P, D_HEAD], F32)
        nc.gpsimd.memset(onesrow_st[:], 0.0)
        nc.gpsimd.memset(onesrow_st[0:1, :], 1.0)
        onesrow = const.tile([P, D_HEAD], F32R)
        nc.scalar.activation(onesrow[:], onesrow_st[:], AF.Copy)
        wqwk_sb = const.tile([D_HEAD, 2], F32)
        nc.sync.dma_start(wqwk_sb[:], wqwk[:])
        eps_t = const.tile([P, 1], F32)
        nc.gpsimd.memset(eps_t[:], EPS)
        wqkv_st = const.tile([P, DM_AUG // P, F_LOCAL], F32)
        nc.sync.dma_start(wqkv_st[:], wqkvt_r)
        wqkv_sb = const.tile([P, DM_AUG // P, F_LOCAL], F32R)
        nc.scalar.activation(wqkv_sb[:], wqkv_st[:], AF.Copy)
        wot_st = const.tile([P, D_MODEL], F32)
        nc.sync.dma_start(wot_st[:], wot[:])
        wot_sb = const.tile([P, D_MODEL], F32R)
        nc.scalar.activation(wot_sb[:], wot_st[:], AF.Copy)

        # attention operand tiles; rows >= 64 of the *hatT tiles stay zero so
        # every matmul contracts over a full 128 partitions
        qhatT = [persist.tile([P, SEQ], F32R, name=f"qhatT{h}", tag=f"qhatT{h}") for h in range(2)]
        khatT = [persist.tile([P, SEQ], F32R, name=f"khatT{h}", tag=f"khatT{h}") for h in range(2)]
        for t in (*qhatT, *khatT):
            for c in range(SEQ // QW):
                nc.scalar.activation(
                    t[D_HEAD:P, bass.ts(c, QW)], zeros_f32[0:D_HEAD, :], AF.Copy
                )
        vp = persist.tile([P, HEADS_LOCAL, KB, D_HEAD + 1], F32R)
        nc.scalar.activation(
            vp[:, :, :, D_HEAD : D_HEAD + 1], ones_f32[:, :, None], AF.Copy
        )
        z_nT = persist.tile([P, SEQ], F32R)
        recip_pad = persist.tile([P, QW], F32R)
        nc.scalar.activation(recip_pad[:], zeros_f32[:], AF.Copy)

        # ---------------- phase 1: QKV + RMSNorm + transposes ----------------
        with ExitStack() as p1:
            xpool = p1.enter_context(tc.tile_pool(name="xt", bufs=3))
            norm = p1.enter_context(tc.tile_pool(name="norm", bufs=2))
            qkps = p1.enter_context(tc.tile_pool(name="qkvps", bufs=2, space="PSUM"))
            tps = p1.enter_context(tc.tile_pool(name="tps", bufs=4, space="PSUM"))

            for sb in range(SB):
                ssl = bass.ts(sb, P)
                xt0 = xpool.tile([P, DM_AUG // P, P], F32)
                nc.sync.dma_start(xt0[:], xta_r[:, :, ssl])
                xt = xpool.tile([P, DM_AUG // P, P], F32R)
                nc.gpsimd.tensor_copy(xt[:], xt0[:])

                qkv_ps = qkps.tile([P, F_LOCAL], F32)
                for c in range(DM_AUG // P):
                    nc.tensor.matmul(
                        qkv_ps[:],
                        lhsT=xt[:, c, :],
                        rhs=wqkv_sb[:, c, :],
                        start=(c == 0),
                        stop=(c == DM_AUG // P - 1),
                    )

                # RMSNorm stats for the 4 (tensor, head) groups of 64
                sq = norm.tile([P, 4, D_HEAD], F32)
                qk_ps = qkv_ps[:, 0 : 4 * D_HEAD].rearrange(
                    "p (g d) -> p g d", g=4
                )
                nc.scalar.activation(sq[:], qk_ps, AF.Square)
                ss = norm.tile([P, 4], F32)
                nc.vector.tensor_reduce(
                    ss[:], sq[:], axis=mybir.AxisListType.X, op=ALU.add
                )
                rs = norm.tile([P, 4], F32)
                nc.scalar.activation(
                    rs[:], ss[:], AF.Sqrt, bias=eps_t[:], scale=1.0 / D_HEAD
                )
                rr = norm.tile([P, 4], F32)
                nc.vector.reciprocal(rr[:], rs[:])

                qk_hat = norm.tile([P, 4, D_HEAD], F32)
                nc.vector.tensor_tensor(
                    qk_hat[:],
                    qk_ps,
                    rr[:, :, None].to_broadcast((P, 4, D_HEAD)),
                    ALU.mult,
                )

                # V chunks for both heads
                nc.vector.tensor_copy(
                    vp[:, :, sb, 0:D_HEAD],
                    qkv_ps[:, 4 * D_HEAD : 6 * D_HEAD].rearrange(
                        "p (h d) -> p h d", h=2
                    ),
                )

                # transposes + wq/wk scaling into [d, s] layout
                for g, (dst, wcol) in enumerate(
                    [(qhatT[0], 0), (qhatT[1], 0), (khatT[0], 1), (khatT[1], 1)]
                ):
                    pt = tps.tile([D_HEAD, P], F32)
                    nc.tensor.transpose(pt[:], qk_hat[:, g, :], ident[:])
                    nc.vector.tensor_scalar_mul(
                        dst[0:D_HEAD, ssl], pt[:], wqwk_sb[:, wcol : wcol + 1]
                    )

        # ---------- phase 2+3: attention with inlined O-projection ----------
        # PSUM: score slots 2x3 banks (shared by O-proj tiles via tag),
        # z' accumulator 1 bank, broadcast 1 bank = 8.
        EXP_BATCH = 3
        with ExitStack() as p2:
            spool = p2.enter_context(tc.tile_pool(name="sps", bufs=2, space="PSUM"))
            zpool = p2.enter_context(tc.tile_pool(name="zps", bufs=1, space="PSUM"))
            bpool = p2.enter_context(tc.tile_pool(name="bps", bufs=1, space="PSUM"))
            ppool = p2.enter_context(tc.tile_pool(name="probs", bufs=3))
            osb = p2.enter_context(tc.tile_pool(name="osb", bufs=2))

            for qt in range(QT):
                qsl = bass.ts(qt, QW)
                for h in range(HEADS_LOCAL):
                    zps = zpool.tile([D_HEAD + 1, QW], F32, name="zps", tag="zps")
                    for kb0 in range(0, KB, EXP_BATCH):
                        nb = min(EXP_BATCH, KB - kb0)
                        sps = spool.tile(
                            [P, EXP_BATCH, QW], F32, name="sps", tag="sps"
                        )
                        for j in range(nb):
                            kb = kb0 + j
                            nc.tensor.matmul(
                                sps[:, j, :],
                                lhsT=khatT[h][:, bass.ts(kb, P)],
                                rhs=qhatT[h][:, qsl],
                                start=True,
                                stop=True,
                            )
                        probs = ppool.tile(
                            [P, EXP_BATCH, QW], F32R, name="probs", tag="probs"
                        )
                        nc.scalar.activation(
                            probs[:, 0:nb, :], sps[:, 0:nb, :], AF.Exp
                        )
                        for j in range(nb):
                            kb = kb0 + j
                            nc.tensor.matmul(
                                zps[:],
                                lhsT=vp[:, h, kb, :],
                                rhs=probs[:, j, :],
                                start=(kb == 0),
                                stop=(kb == KB - 1),
                                skip_group_check=True,
                            )
                    # normalize: z = z / rowsum (rowsum lives in zps row 64)
                    with nc.allow_low_precision(reason="fp32r operand rounding"):
                        nc.vector.reciprocal(
                            recip_pad[0:1, :], zps[D_HEAD : D_HEAD + 1, :]
                        )
                    bps = bpool.tile([D_HEAD, QW], F32, name="bps", tag="bps")
                    nc.tensor.matmul(
                        bps[:],
                        lhsT=onesrow[:],
                        rhs=recip_pad[:],
                        start=True,
                        stop=True,
                    )
                    rb = ppool.tile([D_HEAD, QW], F32, name="rb", tag="rb")
                    nc.vector.tensor_copy(rb[:], bps[:])
                    nc.vector.tensor_tensor(
                        z_nT[h * D_HEAD : (h + 1) * D_HEAD, qsl],
                        zps[0:D_HEAD, :],
                        rb[:],
                        ALU.mult,
                    )
                # O-projection for this q-tile's 4 s-blocks (both heads done);
                # ops tiles share the score-PSUM slots via the "sps" tag
                for sbl in range(QW // P):
                    sb = qt * (QW // P) + sbl
                    ops = spool.tile([P, D_MODEL], F32, name="ops", tag="sps")
                    for half in range(2):
                        nc.tensor.matmul(
                            ops[:, bass.ts(half, QW)],
                            lhsT=z_nT[:, bass.ts(sb, P)],
                            rhs=wot_sb[:, bass.ts(half, QW)],
                            start=True,
                            stop=True,
                        )
                    ot = osb.tile([P, D_MODEL], F32, name="ot", tag="ot")
                    nc.vector.tensor_copy(ot[:], ops[:])
                    nc.sync.dma_start(out[bass.ts(sb, P), :], ot[:])

    if split_waits:
        _split_excess_waits(nc)
    return nc


def shard_inputs(x, Wqkv, bqkv, Wo, bo, wq, wk):
    x2 = np.ascontiguousarray(np.asarray(x, dtype=np.float32).reshape(SEQ, D_MODEL))
    Wqkv = np.asarray(Wqkv, dtype=np.float32)
    bqkv = np.asarray(bqkv, dtype=np.float32)
    Wo = np.asarray(Wo, dtype=np.float32)
    wq = np.asarray(wq, dtype=np.float32)
    wk = np.asarray(wk, dtype=np.float32)

    xta = np.zeros((DM_AUG, SEQ), np.float32)
    xta[:D_MODEL] = x2.T
    xta[D_MODEL] = 1.0
    xta = np.ascontiguousarray(xta)

    wqwk = np.zeros((D_HEAD, 2), np.float32)
    wqwk[:, 0] = wq
    wqwk[:, 1] = wk

    in_maps = []
    for c in range(N_CORES):
        rows, brows = [], []
        for part in range(3):
            for h in (HEADS_LOCAL * c, HEADS_LOCAL * c + 1):
                sl = slice(part * D_MODEL + h * D_HEAD, part * D_MODEL + (h + 1) * D_HEAD)
                rows.append(Wqkv[sl])
                brows.append(bqkv[sl])
        Wl = np.concatenate(rows, 0)          # [384, 1024]
        bl = np.concatenate(brows, 0)         # [384]
        wqkvta = np.zeros((DM_AUG, F_LOCAL), np.float32)
        wqkvta[:D_MODEL] = Wl.T
        wqkvta[D_MODEL] = bl
        cols = slice(HEADS_LOCAL * c * D_HEAD, (HEADS_LOCAL * c + HEADS_LOCAL) * D_HEAD)
        wotc = np.ascontiguousarray(Wo[:, cols].T)    # [128, 1024]
        in_maps.append(
            {
                "xta": xta,
                "wqkvt": np.ascontiguousarray(wqkvta),
                "wot": wotc,
                "wqwk": wqwk,
            }
        )
    return in_maps


_NC_CACHE = {}
LAST_RESULT = None


def kernel(x, Wqkv, bqkv, Wo, bo, wq, wk):
    import os
    from concourse.bass_utils import run_bass_kernel_spmd

    global LAST_RESULT
    assert np.asarray(x).shape == (1, SEQ, D_MODEL)
    in_maps = shard_inputs(x, Wqkv, bqkv, Wo, bo, wq, wk)
    if "nc" not in _NC_CACHE:
        _NC_CACHE["nc"] = build_core_kernel()
    nc = _NC_CACHE["nc"]
    trace = bool(int(os.environ.get("BASS_KERNEL_TRACE", "0")))
    res = run_bass_kernel_spmd(nc, in_maps, list(range(N_CORES)), trace=trace)
    LAST_RESULT = res
    acc = np.zeros((SEQ, D_MODEL), np.float64)
    for c in range(N_CORES):
        acc += res.results[c]["out"].astype(np.float64)
    acc += np.asarray(bo, dtype=np.float64)
    return acc.astype(np.float32).reshape(1, SEQ, D_MODEL)
